# revision 23
# baseline (speedup 1.0000x reference)
"""Trainium2 kernel for nn_M3oE: multi-domain MoE over 26 categorical embeddings.

Sharding: data-parallel over batch across 8 NeuronCores (2048 rows each),
embedding tables replicated in DRAM.

Gather strategy: the SWDGE small-descriptor path costs ~11ns/descriptor
serialized, so per-row (64B) indirect DMAs are the kernel wall.  Instead,
per (tile-group, field) we issue one batched `dma_gather` (Ant ucode) of
512 int16 block-indices, where a block = 4 consecutive table rows (256B,
the dma_gather minimum element).  Calls rotate over 4 SWDGE queues, which
overlaps their drain (~2.6x vs one queue).  The true row within each
gathered 4-row block is selected by a host-built {0,1} mask via a DVE
masked-reduction (y = sum_r S[...,r,:]*M_r), producing the same x layout
the rest of the pipeline used before.

Per-core pipeline (all matmuls in f32r = full-rate fp32):
  1. dma_gather of 512*26 4-row blocks per 512-sample tile group
  2. DVE masked-reduce -> x [128, (t,f,16)]
  3. PE transpose of x chunks -> xT [416(k-chunks), 512]
  4. 8 experts: h1T = relu(W1^T xT + b1) ; h2T = relu(W2^T h1T + b2),
     s_e = Wo . h2T accumulated into one [8, 512] PSUM tile
  5. domain-gated softmax; logits = (sum_e gsel_e * s_e)/denom + bo
"""

import ml_dtypes
import numpy as np

import concourse.bacc as bacc
import concourse.mybir as mybir
import concourse.tile as tile
from concourse.bass_utils import run_bass_kernel_spmd

F = 26
V = 100000
DK = 16
D = 4
E = 8
H1 = 128
H2 = 64
B = 16384
IN = F * DK  # 416
N_CORES = 8
PC = B // N_CORES  # 2048 rows per core
NT = PC // 128  # 16 batch-tiles of 128
NTG = 4  # tile groups
TGW = 512  # columns per tile group
TPG = NT // NTG  # 4 batch-tiles per group
KCH = [(0, 128), (128, 128), (256, 128), (384, 32)]  # k-chunks of IN=416
NQ = 4  # SWDGE queues for dma_gather rotation
BLK = 64  # f32 per gathered block (4 rows x 16)
NBF = V // 4  # 25000 blocks per field
NIX = TPG * 128  # 512 indices per gather call
IXW = NIX // 16  # 32 wrapped idx columns per call

F32 = mybir.dt.float32
F32R = mybir.dt.float32r
BF16 = mybir.dt.bfloat16
I32 = mybir.dt.int32
I16 = mybir.dt.int16

_cache = {}

# test-harness knobs (unused when the harness calls kernel() directly)
TRACE = False
LAST_RESULT = None


def _build(bo_val: float):
    nc = bacc.Bacc("TRN2", target_bir_lowering=False, debug=False,
                   num_devices=N_CORES, num_swdge_queues=NQ)

    emb4 = nc.dram_tensor("emb4", [F * NBF, BLK], F32, kind="ExternalInput")
    idx = nc.dram_tensor("idx16", [128, NTG * F * IXW], I16,
                         kind="ExternalInput")
    msk = nc.dram_tensor("msk", [128, NTG * TPG * F * 4], F32,
                         kind="ExternalInput")
    w1k = [nc.dram_tensor(f"w1k{i}", [w, E * H1], BF16, kind="ExternalInput")
           for i, (_, w) in enumerate(KCH)]
    w2c = nc.dram_tensor("w2c", [H1, E * H2], BF16, kind="ExternalInput")
    wo8 = nc.dram_tensor("wo8", [H2, E * 8], BF16, kind="ExternalInput")
    wgk = [nc.dram_tensor(f"wgk{i}", [w, D * E], BF16, kind="ExternalInput")
           for i, (_, w) in enumerate(KCH)]
    sel8 = nc.dram_tensor("sel8", [D * E, 8], BF16, kind="ExternalInput")
    ones8 = nc.dram_tensor("ones8", [E, 1], BF16, kind="ExternalInput")
    ones32 = nc.dram_tensor("ones32", [D * E, 1], BF16, kind="ExternalInput")
    b1t = nc.dram_tensor("b1t", [H1, E], F32, kind="ExternalInput")
    b2t = nc.dram_tensor("b2t", [H2, E], F32, kind="ExternalInput")
    bgc = nc.dram_tensor("bgc", [D * E, 1], F32, kind="ExternalInput")
    oh = nc.dram_tensor("oh", [D * E, PC], F32, kind="ExternalInput")
    id128 = nc.dram_tensor("id128", [128, 128], BF16, kind="ExternalInput")
    out = nc.dram_tensor("out", [NTG, TGW], F32, kind="ExternalOutput")

    with tile.TileContext(nc) as tc:
        with (
            tc.tile_pool(name="const", bufs=1) as cpool,
            tc.tile_pool(name="stg", bufs=2) as stgpool,
            tc.tile_pool(name="mk", bufs=2) as mkpool,
            tc.tile_pool(name="yv", bufs=2) as ypool,
            tc.tile_pool(name="tmp", bufs=1) as tmppool,
            tc.tile_pool(name="xts", bufs=2 * len(KCH)) as xtspool,
            tc.tile_pool(name="h1s", bufs=3) as h1spool,
            tc.tile_pool(name="h2s", bufs=2) as h2spool,
            tc.tile_pool(name="gsb", bufs=2) as gsbpool,
            tc.tile_pool(name="fin", bufs=1) as finpool,
            tc.tile_pool(name="xtp", bufs=2, space="PSUM") as xtppool,
            tc.tile_pool(name="h1p", bufs=2, space="PSUM") as h1ppool,
            tc.tile_pool(name="h2p", bufs=1, space="PSUM") as h2ppool,
            tc.tile_pool(name="glp", bufs=1, space="PSUM") as glppool,
            tc.tile_pool(name="spp", bufs=1, space="PSUM") as sppool,
        ):
            # --- load constants ---
            def cload(dram, shape, dtype=None):
                t = cpool.tile(shape, dtype or dram.dtype, tag=dram.name)
                nc.sync.dma_start(out=t[:], in_=dram[:])
                return t

            idx_sb = cload(idx, [128, NTG * F * IXW], I16)
            w1_sb = [cload(w1k[i], [w, E * H1]) for i, (_, w) in enumerate(KCH)]
            w2_sb = cload(w2c, [H1, E * H2])
            wo_sb = cload(wo8, [H2, E * 8])
            wg_sb = [cload(wgk[i], [w, D * E]) for i, (_, w) in enumerate(KCH)]
            sel_sb = cload(sel8, [D * E, 8])
            on8_sb = cload(ones8, [E, 1])
            on32_sb = cload(ones32, [D * E, 1])
            b1_sb = cload(b1t, [H1, E])
            b2_sb = cload(b2t, [H2, E])
            bg_sb = cload(bgc, [D * E, 1])
            oh_sb = cload(oh, [D * E, PC])
            id_sb = cload(id128, [128, 128])

            # --- emit ALL gathers first: they stream on the GpSimd queue,
            # paced only by S-buffer recycling (the masked-reduce frees S
            # early).  Compute is then emitted software-pipelined by one
            # group so group g+1's DVE reduce runs during group g's expert
            # matmuls instead of queueing behind g's final logits ops.
            qn = 0
            SM = []
            for tg in range(NTG):
                S = stgpool.tile([128, F * TPG * BLK], F32, tag="S")
                for f in range(F):
                    o0 = f * TPG * BLK
                    nc.gpsimd.dma_gather(
                        out_ap=S[:, o0:o0 + TPG * BLK].rearrange(
                            "p (g e) -> p g e", e=BLK),
                        in_ap=emb4[f * NBF:(f + 1) * NBF, :],
                        idxs_ap=idx_sb[:, (tg * F + f) * IXW:
                                       (tg * F + f + 1) * IXW],
                        num_idxs=NIX,
                        num_idxs_reg=NIX,
                        elem_size=BLK,
                        queue_num=qn % NQ,
                    )
                    qn += 1
                mk = mkpool.tile([128, TPG * F * 4], F32, tag="mk")
                nc.sync.dma_start(
                    out=mk[:],
                    in_=msk[:, tg * TPG * F * 4:(tg + 1) * TPG * F * 4])
                SM.append((S, mk))

            def emit_build_x(tg):
                S, mk = SM[tg]
                # --- select true row from each 4-row block:
                #     y[p,(t,f,d)] = sum_r S[p,(f,t,r,d)] * M[p,(t,f,r)] ---
                Sv = S[:].rearrange("p (f t r d) -> p t f r d",
                                    f=F, t=TPG, r=4, d=DK)
                Mv = mk[:].rearrange("p (t f r) -> p t f r", t=TPG, f=F)
                y = ypool.tile([128, TPG * F * DK], BF16, tag="y")
                t0 = tmppool.tile([128, TPG * F * DK], F32, tag="t0")
                t1 = tmppool.tile([128, TPG * F * DK], F32, tag="t1")

                def rsel(r):
                    return (Sv[:, :, :, r, :],
                            Mv[:, :, :, r:r + 1].broadcast_to(
                                [128, TPG, F, DK]))

                s0, m0 = rsel(0)
                nc.vector.tensor_tensor(out=t0[:], in0=s0, in1=m0,
                                        op=mybir.AluOpType.mult)
                s1, m1 = rsel(1)
                nc.vector.tensor_tensor(out=t1[:], in0=s1, in1=m1,
                                        op=mybir.AluOpType.mult)
                nc.vector.tensor_tensor(out=t0[:], in0=t0[:], in1=t1[:],
                                        op=mybir.AluOpType.add)
                s2, m2 = rsel(2)
                nc.vector.tensor_tensor(out=t1[:], in0=s2, in1=m2,
                                        op=mybir.AluOpType.mult)
                nc.vector.tensor_tensor(out=t0[:], in0=t0[:], in1=t1[:],
                                        op=mybir.AluOpType.add)
                s3, m3 = rsel(3)
                nc.vector.tensor_tensor(out=t1[:], in0=s3, in1=m3,
                                        op=mybir.AluOpType.mult)
                nc.vector.tensor_tensor(out=y[:], in0=t0[:], in1=t1[:],
                                        op=mybir.AluOpType.add)

                # --- transpose x -> xT per k-chunk (PE), evict to SBUF ---
                xts = []
                for kc, (koff, kw) in enumerate(KCH):
                    xtp = xtppool.tile([128, TGW], BF16, space="PSUM", tag="xtp")
                    for tl in range(TPG):
                        nc.tensor.transpose(
                            out=xtp[0:kw, tl * 128:(tl + 1) * 128],
                            in_=y[:, tl * IN + koff: tl * IN + koff + kw],
                            identity=id_sb[:],
                        )
                    xt = xtspool.tile([kw, TGW], BF16, tag=f"xts{kc}")
                    nc.vector.tensor_copy(xt[:], xtp[0:kw, :])
                    xts.append(xt)
                return xts

            def emit_C(tg, xts):
                # --- gating ---
                glp = glppool.tile([D * E, TGW], F32, space="PSUM", tag="glp")
                for kc in range(len(KCH)):
                    nc.tensor.matmul(glp[:], wg_sb[kc][:], xts[kc][:],
                                     start=(kc == 0), stop=(kc == len(KCH) - 1))
                expsb = gsbpool.tile([D * E, TGW], F32, tag="expsb")
                nc.scalar.activation(expsb[:], glp[:],
                                     mybir.ActivationFunctionType.Exp,
                                     bias=bg_sb[:, 0:1])
                masked = gsbpool.tile([D * E, TGW], BF16, tag="masked")
                nc.vector.tensor_tensor(
                    out=masked[:], in0=expsb[:],
                    in1=oh_sb[:, tg * TGW:(tg + 1) * TGW],
                    op=mybir.AluOpType.mult)
                gslp = glppool.tile([D * E, TGW], F32, space="PSUM", tag="glp")
                nc.tensor.matmul(gslp[0:8, :], sel_sb[:], masked[:],
                                 start=True, stop=True)
                gssb = gsbpool.tile([8, TGW], F32, tag="gssb")
                nc.scalar.activation(gssb[:], gslp[0:8, :],
                                     mybir.ActivationFunctionType.Copy)

                # --- experts ---
                sp = sppool.tile([E, 2 * TGW], F32, space="PSUM", tag="spp")
                for e in range(E):
                    h1p = h1ppool.tile([H1, TGW], F32, space="PSUM", tag="h1p")
                    for kc in range(len(KCH)):
                        nc.tensor.matmul(
                            h1p[:], w1_sb[kc][:, e * H1:(e + 1) * H1],
                            xts[kc][:],
                            start=(kc == 0), stop=(kc == len(KCH) - 1))
                    h1s = h1spool.tile([H1, TGW], BF16, tag="h1s")
                    nc.scalar.activation(h1s[:], h1p[:],
                                         mybir.ActivationFunctionType.Relu,
                                         bias=b1_sb[:, e:e + 1])
                    h2p = h2ppool.tile([H2, TGW], F32, space="PSUM", tag="h2p")
                    nc.tensor.matmul(h2p[:], w2_sb[:, e * H2:(e + 1) * H2],
                                     h1s[:], start=True, stop=True)
                    h2s = h2spool.tile([H2, TGW], BF16, tag="h2s")
                    nc.scalar.activation(h2s[:], h2p[:],
                                         mybir.ActivationFunctionType.Relu,
                                         bias=b2_sb[:, e:e + 1])
                    nc.tensor.matmul(sp[:, 0:TGW],
                                     wo_sb[:, e * 8:(e + 1) * 8], h2s[:],
                                     start=(e == 0), stop=(e == E - 1),
                                     skip_group_check=True)
                return masked, gssb, sp

            def emit_D(tg, masked, gssb, sp):
                # --- final: logits = (sum_e gsel*s)/denom + bo ---
                msb = finpool.tile([E, TGW], BF16, tag="msb")
                nc.vector.tensor_tensor(out=msb[:], in0=sp[:, 0:TGW],
                                        in1=gssb[:],
                                        op=mybir.AluOpType.mult)
                updn = sppool.tile([E, 2 * TGW], F32, space="PSUM", tag="spp")
                nc.tensor.matmul(updn[0:1, 0:TGW], on8_sb[:], msb[:],
                                 start=True, stop=True)
                nc.tensor.matmul(updn[0:1, TGW:2 * TGW], on32_sb[:], masked[:],
                                 start=True, stop=True)
                rr = finpool.tile([1, TGW], F32, tag="rr")
                nc.vector.reciprocal(rr[:], updn[0:1, TGW:2 * TGW])
                lsb = finpool.tile([1, TGW], F32, tag="lsb")
                nc.vector.tensor_tensor(out=lsb[:], in0=updn[0:1, 0:TGW],
                                        in1=rr[:], op=mybir.AluOpType.mult)
                nc.vector.tensor_scalar_add(lsb[:], lsb[:], float(bo_val))
                nc.sync.dma_start(out=out[tg:tg + 1, :], in_=lsb[:])

            # emit order per group: build_x(g), finals(g-1), heavy
            # compute(g).  The next group's DVE reduce thus runs during
            # this group's expert matmuls; only the cheap final-logits ops
            # queue behind it, and gating is never gated on a later
            # group's gathers.
            prevD = None
            for tg in range(NTG):
                xts = emit_build_x(tg)
                if prevD is not None:
                    emit_D(*prevD)
                prevD = (tg, *emit_C(tg, xts))
            emit_D(*prevD)

    nc.compile()
    return nc


def kernel(**inputs):
    features = np.asarray(inputs["features"])
    domain = np.asarray(inputs["domain_indicator"])
    emb = np.asarray(inputs["emb"], dtype=np.float32)
    W1 = np.asarray(inputs["W1"], dtype=np.float32)
    b1 = np.asarray(inputs["b1"], dtype=np.float32)
    W2 = np.asarray(inputs["W2"], dtype=np.float32)
    b2 = np.asarray(inputs["b2"], dtype=np.float32)
    Wg = np.asarray(inputs["Wg"], dtype=np.float32)
    bg = np.asarray(inputs["bg"], dtype=np.float32)
    Wo = np.asarray(inputs["Wo"], dtype=np.float32)
    bo = np.asarray(inputs["bo"], dtype=np.float32)

    bo_val = float(bo.reshape(-1)[0])
    key = ("m3oe", bo_val)
    if key not in _cache:
        _cache[key] = _build(bo_val)
    nc = _cache[key]

    # ---- host-side prep (shared across cores) ----
    emb4_np = np.ascontiguousarray(emb.reshape(F * NBF, BLK))

    w1k = []
    wgk = []
    for koff, kw in KCH:
        w1k.append(np.ascontiguousarray(
            W1[:, koff:koff + kw, :].transpose(1, 0, 2).reshape(kw, E * H1)))
        wgk.append(np.ascontiguousarray(
            Wg[:, koff:koff + kw, :].transpose(1, 0, 2).reshape(kw, D * E)))
    w2c = np.ascontiguousarray(W2.transpose(1, 0, 2).reshape(H1, E * H2))
    wo8 = np.zeros((H2, E * 8), dtype=np.float32)
    wov = Wo.reshape(H2)
    for e in range(E):
        wo8[:, e * 8 + e] = wov
    sel8 = np.zeros((D * E, 8), dtype=np.float32)
    for d in range(D):
        for e in range(E):
            sel8[d * 8 + e, e] = 1.0
    ones8 = np.ones((E, 1), dtype=np.float32)
    ones32 = np.ones((D * E, 1), dtype=np.float32)
    b1t = np.ascontiguousarray(b1.T)  # [H1, E]
    b2t = np.ascontiguousarray(b2.T)  # [H2, E]
    bgc = bg.reshape(D * E, 1).astype(np.float32)
    id128 = np.eye(128, dtype=np.float32)

    bf = ml_dtypes.bfloat16
    shared = {
        "emb4": emb4_np,
        "w2c": w2c.astype(bf), "wo8": wo8.astype(bf),
        "sel8": sel8.astype(bf), "ones8": ones8.astype(bf),
        "ones32": ones32.astype(bf), "b1t": b1t, "b2t": b2t,
        "bgc": bgc, "id128": id128.astype(bf),
    }
    for i in range(len(KCH)):
        shared[f"w1k{i}"] = w1k[i].astype(bf)
        shared[f"wgk{i}"] = wgk[i].astype(bf)

    derep = np.repeat(np.arange(D), E)  # [32] domain of each (d,e) row
    lane = np.arange(128) % 16
    in_maps = []
    for c in range(N_CORES):
        sl = slice(c * PC, (c + 1) * PC)
        fc = features[sl].astype(np.int64)  # [PC, F]
        blk = (fc // 4).astype(np.int16)    # block index within field
        res = (fc % 4).astype(np.int64)     # row slot within block

        # idx16[p, (tg*F+f)*IXW + s] = blk[(tg*TPG+t)*128+p', f] where the
        # gather-order position i = t*128+p' is stored wrapped:
        # value at (p, s) is position s*16 + p%16.
        idx_core = np.zeros((128, NTG * F * IXW), dtype=np.int16)
        # blk reshaped per group: [NTG, TPG*128, F] -> positions i = t*128+p'
        blkg = blk.reshape(NTG, NIX, F)
        for tg in range(NTG):
            # w[s, lane, f] = blkg[tg, s*16+lane, f]
            w = blkg[tg].reshape(IXW, 16, F)
            # idx_core[p, ...] = w[s, p%16, f]
            block = w[:, lane, :]            # [IXW, 128, F]
            block = block.transpose(1, 2, 0)  # [128, F, IXW]
            idx_core[:, tg * F * IXW:(tg + 1) * F * IXW] = \
                block.reshape(128, F * IXW)

        # msk[p, (tg, t, f, r)]: 1.0 where sample (tg,t,p)'s field-f row
        # sits at slot r of its 4-row block
        resg = res.reshape(NTG, TPG, 128, F)  # [tg, t, p, f]
        onehot = (resg[:, :, :, :, None] ==
                  np.arange(4)).astype(np.float32)  # [tg, t, p, f, r]
        mk = np.ascontiguousarray(
            onehot.transpose(2, 0, 1, 3, 4).reshape(
                128, NTG * TPG * F * 4))

        dom = domain[sl].astype(np.int64)
        oh_core = (dom[None, :] == derep[:, None]).astype(np.float32)
        m = dict(shared)
        m["idx16"] = idx_core
        m["msk"] = mk
        m["oh"] = oh_core
        in_maps.append(m)

    global LAST_RESULT
    res_k = run_bass_kernel_spmd(nc, in_maps, core_ids=list(range(N_CORES)),
                                 trace=TRACE)
    LAST_RESULT = res_k
    outs = [res_k.results[c]["out"].reshape(PC) for c in range(N_CORES)]
    return np.concatenate(outs).astype(np.float32)


# revision 24
# speedup vs baseline: 1.0029x; 1.0029x over previous
"""Trainium2 kernel for nn_M3oE: multi-domain MoE over 26 categorical embeddings.

Sharding: data-parallel over batch across 8 NeuronCores (2048 rows each),
embedding tables replicated in DRAM.

Gather strategy: the SWDGE small-descriptor path costs ~11ns/descriptor
serialized, so per-row (64B) indirect DMAs are the kernel wall.  Instead,
per (tile-group, field) we issue one batched `dma_gather` (Ant ucode) of
512 int16 block-indices, where a block = 4 consecutive table rows (256B,
the dma_gather minimum element).  Calls rotate over 4 SWDGE queues, which
overlaps their drain (~2.6x vs one queue).  The true row within each
gathered 4-row block is selected by a host-built {0,1} mask via a DVE
masked-reduction (y = sum_r S[...,r,:]*M_r), producing the same x layout
the rest of the pipeline used before.

Per-core pipeline (all matmuls in f32r = full-rate fp32):
  1. dma_gather of 512*26 4-row blocks per 512-sample tile group
  2. DVE masked-reduce -> x [128, (t,f,16)]
  3. PE transpose of x chunks -> xT [416(k-chunks), 512]
  4. 8 experts: h1T = relu(W1^T xT + b1) ; h2T = relu(W2^T h1T + b2),
     s_e = Wo . h2T accumulated into one [8, 512] PSUM tile
  5. domain-gated softmax; logits = (sum_e gsel_e * s_e)/denom + bo
"""

import ml_dtypes
import numpy as np

import concourse.bacc as bacc
import concourse.mybir as mybir
import concourse.tile as tile
from concourse.bass_utils import run_bass_kernel_spmd

F = 26
V = 100000
DK = 16
D = 4
E = 8
H1 = 128
H2 = 64
B = 16384
IN = F * DK  # 416
N_CORES = 8
PC = B // N_CORES  # 2048 rows per core
NT = PC // 128  # 16 batch-tiles of 128
NTG = 4  # tile groups
TGW = 512  # columns per tile group
TPG = NT // NTG  # 4 batch-tiles per group
KCH = [(0, 128), (128, 128), (256, 128), (384, 32)]  # k-chunks of IN=416
NQ = 4  # SWDGE queues for dma_gather rotation
BLK = 64  # f32 per gathered block (4 rows x 16)
NBF = V // 4  # 25000 blocks per field
NIX = TPG * 128  # 512 indices per gather call
IXW = NIX // 16  # 32 wrapped idx columns per call

F32 = mybir.dt.float32
F32R = mybir.dt.float32r
BF16 = mybir.dt.bfloat16
I32 = mybir.dt.int32
I16 = mybir.dt.int16

_cache = {}

# test-harness knobs (unused when the harness calls kernel() directly)
TRACE = False
LAST_RESULT = None


def _build(bo_val: float):
    nc = bacc.Bacc("TRN2", target_bir_lowering=False, debug=False,
                   num_devices=N_CORES, num_swdge_queues=NQ)

    emb4 = nc.dram_tensor("emb4", [F * NBF, BLK], F32, kind="ExternalInput")
    idx = nc.dram_tensor("idx16", [128, NTG * F * IXW], I16,
                         kind="ExternalInput")
    msk = nc.dram_tensor("msk", [128, NTG * TPG * F * 4], F32,
                         kind="ExternalInput")
    w1k = [nc.dram_tensor(f"w1k{i}", [w, E * H1], BF16, kind="ExternalInput")
           for i, (_, w) in enumerate(KCH)]
    w2c = nc.dram_tensor("w2c", [H1, E * H2], BF16, kind="ExternalInput")
    wo8 = nc.dram_tensor("wo8", [H2, E * 8], BF16, kind="ExternalInput")
    wgk = [nc.dram_tensor(f"wgk{i}", [w, D * E], BF16, kind="ExternalInput")
           for i, (_, w) in enumerate(KCH)]
    sel8 = nc.dram_tensor("sel8", [D * E, 8], BF16, kind="ExternalInput")
    ones8 = nc.dram_tensor("ones8", [E, 1], BF16, kind="ExternalInput")
    ones32 = nc.dram_tensor("ones32", [D * E, 1], BF16, kind="ExternalInput")
    b1t = nc.dram_tensor("b1t", [H1, E], F32, kind="ExternalInput")
    b2t = nc.dram_tensor("b2t", [H2, E], F32, kind="ExternalInput")
    bgc = nc.dram_tensor("bgc", [D * E, 1], F32, kind="ExternalInput")
    oh = nc.dram_tensor("oh", [D * E, PC], F32, kind="ExternalInput")
    id128 = nc.dram_tensor("id128", [128, 128], BF16, kind="ExternalInput")
    out = nc.dram_tensor("out", [NTG, TGW], F32, kind="ExternalOutput")

    with tile.TileContext(nc) as tc:
        with (
            tc.tile_pool(name="const", bufs=1) as cpool,
            tc.tile_pool(name="stg", bufs=2) as stgpool,
            tc.tile_pool(name="mk", bufs=2) as mkpool,
            tc.tile_pool(name="yv", bufs=2) as ypool,
            tc.tile_pool(name="tmp", bufs=1) as tmppool,
            tc.tile_pool(name="xts", bufs=2 * len(KCH)) as xtspool,
            tc.tile_pool(name="h1s", bufs=3) as h1spool,
            tc.tile_pool(name="h2s", bufs=2) as h2spool,
            tc.tile_pool(name="gsb", bufs=2) as gsbpool,
            tc.tile_pool(name="fin", bufs=1) as finpool,
            tc.tile_pool(name="xtp", bufs=2, space="PSUM") as xtppool,
            tc.tile_pool(name="h1p", bufs=2, space="PSUM") as h1ppool,
            tc.tile_pool(name="h2p", bufs=1, space="PSUM") as h2ppool,
            tc.tile_pool(name="glp", bufs=1, space="PSUM") as glppool,
            tc.tile_pool(name="spp", bufs=1, space="PSUM") as sppool,
        ):
            # --- load constants ---
            def cload(dram, shape, dtype=None):
                t = cpool.tile(shape, dtype or dram.dtype, tag=dram.name)
                nc.sync.dma_start(out=t[:], in_=dram[:])
                return t

            idx_sb = cload(idx, [128, NTG * F * IXW], I16)
            w1_sb = [cload(w1k[i], [w, E * H1]) for i, (_, w) in enumerate(KCH)]
            w2_sb = cload(w2c, [H1, E * H2])
            wo_sb = cload(wo8, [H2, E * 8])
            wg_sb = [cload(wgk[i], [w, D * E]) for i, (_, w) in enumerate(KCH)]
            sel_sb = cload(sel8, [D * E, 8])
            on8_sb = cload(ones8, [E, 1])
            on32_sb = cload(ones32, [D * E, 1])
            b1_sb = cload(b1t, [H1, E])
            b2_sb = cload(b2t, [H2, E])
            bg_sb = cload(bgc, [D * E, 1])
            oh_sb = cload(oh, [D * E, PC])
            id_sb = cload(id128, [128, 128])

            # --- emit ALL gathers first: they stream on the GpSimd queue,
            # paced only by S-buffer recycling (the masked-reduce frees S
            # early).  Compute is then emitted software-pipelined by one
            # group so group g+1's DVE reduce runs during group g's expert
            # matmuls instead of queueing behind g's final logits ops.
            qn = 0
            SM = []
            for tg in range(NTG):
                S = stgpool.tile([128, F * TPG * BLK], F32, tag="S")
                for f in range(F):
                    o0 = f * TPG * BLK
                    nc.gpsimd.dma_gather(
                        out_ap=S[:, o0:o0 + TPG * BLK].rearrange(
                            "p (g e) -> p g e", e=BLK),
                        in_ap=emb4[f * NBF:(f + 1) * NBF, :],
                        idxs_ap=idx_sb[:, (tg * F + f) * IXW:
                                       (tg * F + f + 1) * IXW],
                        num_idxs=NIX,
                        num_idxs_reg=NIX,
                        elem_size=BLK,
                        queue_num=qn % NQ,
                    )
                    qn += 1
                mk = mkpool.tile([128, TPG * F * 4], F32, tag="mk")
                nc.sync.dma_start(
                    out=mk[:],
                    in_=msk[:, tg * TPG * F * 4:(tg + 1) * TPG * F * 4])
                SM.append((S, mk))

            def emit_build_x(tg):
                S, mk = SM[tg]
                # --- select true row from each 4-row block:
                #     y[p,(t,f,d)] = sum_r S[p,(f,t,r,d)] * M[p,(t,f,r)] ---
                Sv = S[:].rearrange("p (f t r d) -> p t f r d",
                                    f=F, t=TPG, r=4, d=DK)
                Mv = mk[:].rearrange("p (t f r) -> p t f r", t=TPG, f=F)
                y = ypool.tile([128, TPG * F * DK], BF16, tag="y")
                t0 = tmppool.tile([128, TPG * F * DK], F32, tag="t0")
                t1 = tmppool.tile([128, TPG * F * DK], F32, tag="t1")

                def rsel(r):
                    return (Sv[:, :, :, r, :],
                            Mv[:, :, :, r:r + 1].broadcast_to(
                                [128, TPG, F, DK]))

                s0, m0 = rsel(0)
                nc.vector.tensor_tensor(out=t0[:], in0=s0, in1=m0,
                                        op=mybir.AluOpType.mult)
                s1, m1 = rsel(1)
                nc.vector.tensor_tensor(out=t1[:], in0=s1, in1=m1,
                                        op=mybir.AluOpType.mult)
                nc.vector.tensor_tensor(out=t0[:], in0=t0[:], in1=t1[:],
                                        op=mybir.AluOpType.add)
                s2, m2 = rsel(2)
                nc.vector.tensor_tensor(out=t1[:], in0=s2, in1=m2,
                                        op=mybir.AluOpType.mult)
                nc.vector.tensor_tensor(out=t0[:], in0=t0[:], in1=t1[:],
                                        op=mybir.AluOpType.add)
                s3, m3 = rsel(3)
                nc.vector.tensor_tensor(out=t1[:], in0=s3, in1=m3,
                                        op=mybir.AluOpType.mult)
                nc.vector.tensor_tensor(out=y[:], in0=t0[:], in1=t1[:],
                                        op=mybir.AluOpType.add)

                # --- transpose x -> xT per k-chunk (PE), evict to SBUF ---
                xts = []
                for kc, (koff, kw) in enumerate(KCH):
                    xtp = xtppool.tile([128, TGW], BF16, space="PSUM", tag="xtp")
                    for tl in range(TPG):
                        nc.tensor.transpose(
                            out=xtp[0:kw, tl * 128:(tl + 1) * 128],
                            in_=y[:, tl * IN + koff: tl * IN + koff + kw],
                            identity=id_sb[:],
                        )
                    xt = xtspool.tile([kw, TGW], BF16, tag=f"xts{kc}")
                    nc.vector.tensor_copy(xt[:], xtp[0:kw, :])
                    xts.append(xt)
                return xts

            def emit_C(tg, xts):
                # --- gating ---
                glp = glppool.tile([D * E, TGW], F32, space="PSUM", tag="glp")
                for kc in range(len(KCH)):
                    nc.tensor.matmul(glp[:], wg_sb[kc][:], xts[kc][:],
                                     start=(kc == 0), stop=(kc == len(KCH) - 1))
                expsb = gsbpool.tile([D * E, TGW], F32, tag="expsb")
                nc.scalar.activation(expsb[:], glp[:],
                                     mybir.ActivationFunctionType.Exp,
                                     bias=bg_sb[:, 0:1])
                masked = gsbpool.tile([D * E, TGW], BF16, tag="masked")
                nc.vector.tensor_tensor(
                    out=masked[:], in0=expsb[:],
                    in1=oh_sb[:, tg * TGW:(tg + 1) * TGW],
                    op=mybir.AluOpType.mult)
                gslp = glppool.tile([D * E, TGW], F32, space="PSUM", tag="glp")
                nc.tensor.matmul(gslp[0:8, :], sel_sb[:], masked[:],
                                 start=True, stop=True)
                gssb = gsbpool.tile([8, TGW], F32, tag="gssb")
                nc.scalar.activation(gssb[:], gslp[0:8, :],
                                     mybir.ActivationFunctionType.Copy)

                # --- experts ---
                sp = sppool.tile([E, 2 * TGW], F32, space="PSUM", tag="spp")
                for e in range(E):
                    h1p = h1ppool.tile([H1, TGW], F32, space="PSUM", tag="h1p")
                    for kc in range(len(KCH)):
                        nc.tensor.matmul(
                            h1p[:], w1_sb[kc][:, e * H1:(e + 1) * H1],
                            xts[kc][:],
                            start=(kc == 0), stop=(kc == len(KCH) - 1))
                    h1s = h1spool.tile([H1, TGW], BF16, tag="h1s")
                    nc.scalar.activation(h1s[:], h1p[:],
                                         mybir.ActivationFunctionType.Relu,
                                         bias=b1_sb[:, e:e + 1])
                    h2p = h2ppool.tile([H2, TGW], F32, space="PSUM", tag="h2p")
                    nc.tensor.matmul(h2p[:], w2_sb[:, e * H2:(e + 1) * H2],
                                     h1s[:], start=True, stop=True)
                    h2s = h2spool.tile([H2, TGW], BF16, tag="h2s")
                    nc.scalar.activation(h2s[:], h2p[:],
                                         mybir.ActivationFunctionType.Relu,
                                         bias=b2_sb[:, e:e + 1])
                    nc.tensor.matmul(sp[:, 0:TGW],
                                     wo_sb[:, e * 8:(e + 1) * 8], h2s[:],
                                     start=(e == 0), stop=(e == E - 1),
                                     skip_group_check=True)
                return masked, gssb, sp

            def emit_D(tg, masked, gssb, sp):
                # --- final: logits = (sum_e gsel*s)/denom + bo ---
                msb = finpool.tile([E, TGW], BF16, tag="msb")
                nc.vector.tensor_tensor(out=msb[:], in0=sp[:, 0:TGW],
                                        in1=gssb[:],
                                        op=mybir.AluOpType.mult)
                updn = sppool.tile([E, 2 * TGW], F32, space="PSUM", tag="spp")
                nc.tensor.matmul(updn[0:1, 0:TGW], on8_sb[:], msb[:],
                                 start=True, stop=True)
                nc.tensor.matmul(updn[0:1, TGW:2 * TGW], on32_sb[:], masked[:],
                                 start=True, stop=True)
                rr = finpool.tile([1, TGW], F32, tag="rr")
                nc.vector.reciprocal(rr[:], updn[0:1, TGW:2 * TGW])
                lsb = finpool.tile([1, TGW], F32, tag="lsb")
                nc.vector.tensor_tensor(out=lsb[:], in0=updn[0:1, 0:TGW],
                                        in1=rr[:], op=mybir.AluOpType.mult)
                nc.vector.tensor_scalar_add(lsb[:], lsb[:], float(bo_val))
                nc.sync.dma_start(out=out[tg:tg + 1, :], in_=lsb[:])

            # Serial per-group emission measured fastest (222.7us): the
            # gather stream (emitted fully upfront) runs ahead on its own
            # queues; finer software-pipelining of compute phases was
            # tried and lost to SBUF/engine contention.
            for tg in range(NTG):
                xts = emit_build_x(tg)
                emit_D(tg, *emit_C(tg, xts))

    nc.compile()
    return nc


def kernel(**inputs):
    features = np.asarray(inputs["features"])
    domain = np.asarray(inputs["domain_indicator"])
    emb = np.asarray(inputs["emb"], dtype=np.float32)
    W1 = np.asarray(inputs["W1"], dtype=np.float32)
    b1 = np.asarray(inputs["b1"], dtype=np.float32)
    W2 = np.asarray(inputs["W2"], dtype=np.float32)
    b2 = np.asarray(inputs["b2"], dtype=np.float32)
    Wg = np.asarray(inputs["Wg"], dtype=np.float32)
    bg = np.asarray(inputs["bg"], dtype=np.float32)
    Wo = np.asarray(inputs["Wo"], dtype=np.float32)
    bo = np.asarray(inputs["bo"], dtype=np.float32)

    bo_val = float(bo.reshape(-1)[0])
    key = ("m3oe", bo_val)
    if key not in _cache:
        _cache[key] = _build(bo_val)
    nc = _cache[key]

    # ---- host-side prep (shared across cores) ----
    emb4_np = np.ascontiguousarray(emb.reshape(F * NBF, BLK))

    w1k = []
    wgk = []
    for koff, kw in KCH:
        w1k.append(np.ascontiguousarray(
            W1[:, koff:koff + kw, :].transpose(1, 0, 2).reshape(kw, E * H1)))
        wgk.append(np.ascontiguousarray(
            Wg[:, koff:koff + kw, :].transpose(1, 0, 2).reshape(kw, D * E)))
    w2c = np.ascontiguousarray(W2.transpose(1, 0, 2).reshape(H1, E * H2))
    wo8 = np.zeros((H2, E * 8), dtype=np.float32)
    wov = Wo.reshape(H2)
    for e in range(E):
        wo8[:, e * 8 + e] = wov
    sel8 = np.zeros((D * E, 8), dtype=np.float32)
    for d in range(D):
        for e in range(E):
            sel8[d * 8 + e, e] = 1.0
    ones8 = np.ones((E, 1), dtype=np.float32)
    ones32 = np.ones((D * E, 1), dtype=np.float32)
    b1t = np.ascontiguousarray(b1.T)  # [H1, E]
    b2t = np.ascontiguousarray(b2.T)  # [H2, E]
    bgc = bg.reshape(D * E, 1).astype(np.float32)
    id128 = np.eye(128, dtype=np.float32)

    bf = ml_dtypes.bfloat16
    shared = {
        "emb4": emb4_np,
        "w2c": w2c.astype(bf), "wo8": wo8.astype(bf),
        "sel8": sel8.astype(bf), "ones8": ones8.astype(bf),
        "ones32": ones32.astype(bf), "b1t": b1t, "b2t": b2t,
        "bgc": bgc, "id128": id128.astype(bf),
    }
    for i in range(len(KCH)):
        shared[f"w1k{i}"] = w1k[i].astype(bf)
        shared[f"wgk{i}"] = wgk[i].astype(bf)

    derep = np.repeat(np.arange(D), E)  # [32] domain of each (d,e) row
    lane = np.arange(128) % 16
    in_maps = []
    for c in range(N_CORES):
        sl = slice(c * PC, (c + 1) * PC)
        fc = features[sl].astype(np.int64)  # [PC, F]
        blk = (fc // 4).astype(np.int16)    # block index within field
        res = (fc % 4).astype(np.int64)     # row slot within block

        # idx16[p, (tg*F+f)*IXW + s] = blk[(tg*TPG+t)*128+p', f] where the
        # gather-order position i = t*128+p' is stored wrapped:
        # value at (p, s) is position s*16 + p%16.
        idx_core = np.zeros((128, NTG * F * IXW), dtype=np.int16)
        # blk reshaped per group: [NTG, TPG*128, F] -> positions i = t*128+p'
        blkg = blk.reshape(NTG, NIX, F)
        for tg in range(NTG):
            # w[s, lane, f] = blkg[tg, s*16+lane, f]
            w = blkg[tg].reshape(IXW, 16, F)
            # idx_core[p, ...] = w[s, p%16, f]
            block = w[:, lane, :]            # [IXW, 128, F]
            block = block.transpose(1, 2, 0)  # [128, F, IXW]
            idx_core[:, tg * F * IXW:(tg + 1) * F * IXW] = \
                block.reshape(128, F * IXW)

        # msk[p, (tg, t, f, r)]: 1.0 where sample (tg,t,p)'s field-f row
        # sits at slot r of its 4-row block
        resg = res.reshape(NTG, TPG, 128, F)  # [tg, t, p, f]
        onehot = (resg[:, :, :, :, None] ==
                  np.arange(4)).astype(np.float32)  # [tg, t, p, f, r]
        mk = np.ascontiguousarray(
            onehot.transpose(2, 0, 1, 3, 4).reshape(
                128, NTG * TPG * F * 4))

        dom = domain[sl].astype(np.int64)
        oh_core = (dom[None, :] == derep[:, None]).astype(np.float32)
        m = dict(shared)
        m["idx16"] = idx_core
        m["msk"] = mk
        m["oh"] = oh_core
        in_maps.append(m)

    global LAST_RESULT
    res_k = run_bass_kernel_spmd(nc, in_maps, core_ids=list(range(N_CORES)),
                                 trace=TRACE)
    LAST_RESULT = res_k
    outs = [res_k.results[c]["out"].reshape(PC) for c in range(N_CORES)]
    return np.concatenate(outs).astype(np.float32)
